# revision 17
# baseline (speedup 1.0000x reference)
"""nn_Attention_50749333569807 — 8-core Trainium2 Bass kernel.

Channel attention (XCA-style) over x[4, 384, 128, 128]:
  qkv = 1x1 conv -> depthwise 3x3 -> per-head (8) channel attention with
  L2-normalized q,k over the spatial axis -> 1x1 projection.

Sharding: H (128 rows) split into 8 slabs of 16 rows, one per NeuronCore.
All 4 batches + all channels live on every core.  The depthwise conv uses a
1-row halo (host-supplied, zero padded at image edges).  The only cross-core
coupling is the spatial contraction inside the q/k grams and norms; that is
one AllReduce of the 32 per-(batch,head) gram blocks (2 MB).

Layout notes (HW: engine APs need partition base % 32 == 0; matmul operands
need equal bases in {0,32,64}):
  * gram blocks are [128,128]: q channels at rows/cols [0,48), k at [64,112),
    by reading 64-wide per-head windows of the transposed q|k tiles.
  * v and the attention output use a "head pair" layout: tile hp holds head
    2hp at partitions [0,48) and head 2hp+1 at [64,112).  v is repacked from
    the natural eviction layout via partition-offset DMAs (DMA has no
    alignment restriction); the projection weights are host-permuted to
    match, with zero rows at the padding.

Per-core pipeline (per batch):
  1. qkv GEMM  [1152,384]@[384,2304]  (bf16, f32 psum), evicted into a
     width-130 padded layout (explicit zero columns for x-edge padding).
  2. depthwise 3x3 = 9 accumulating matmuls with diagonal-stationary
     weights against shifted slices of the padded qkv.
  3. q,k tiles PE-transposed to [n, c] layout (bf16).
  4. gram[h] accumulated over 16 K-tiles of 128 pixels -> staged to DRAM.
  -- AllReduce over 8 cores --
  5. norms from the gram diagonal; softmax scaling folded as
     attn = softmax(G_qk * inv_q[c] * inv_k[d] * temp); 1/Z folded into
     the attn@v eviction.
  6. out = attn @ v per head; final = w_proj @ out; DMA to output.
"""

import sys

import numpy as np

if "/opt/trn_rl_repo" not in sys.path:
    sys.path.insert(0, "/opt/trn_rl_repo")

import ml_dtypes

BF16 = ml_dtypes.bfloat16

B, DIM, H, W = 4, 384, 128, 128
HEADS = 8
CH = DIM // HEADS            # 48
N_CORES = 8
ROWS = H // N_CORES          # 16 valid rows per core
HR = ROWS + 2                # 18 rows incl halo
PITCH = W + 2                # 130 padded row pitch
NV = ROWS * W                # 2048 valid pixels / core / batch
NH = HR * W                  # 2304 pixels incl halo
EPS = 1e-12

_CACHE = {}


def _build():
    import concourse.tile as tile
    from concourse import bacc, mybir

    dt = mybir.dt
    nc = bacc.Bacc("TRN2", target_bir_lowering=False, debug=False,
                   num_devices=N_CORES)

    x_d = nc.dram_tensor("x", [B, DIM, NH], dt.bfloat16, kind="ExternalInput").ap()
    wqkvT_d = nc.dram_tensor("wqkvT", [DIM, 3 * DIM], dt.bfloat16, kind="ExternalInput").ap()
    wdw_d = nc.dram_tensor("wdw", [128, 81], dt.float32, kind="ExternalInput").ap()
    wprojT_d = nc.dram_tensor("wprojT", [4 * 128, DIM], dt.bfloat16, kind="ExternalInput").ap()
    ident_d = nc.dram_tensor("ident", [128, 128], dt.bfloat16, kind="ExternalInput").ap()
    id48_d = nc.dram_tensor("id48", [48, 48], dt.float32, kind="ExternalInput").ap()
    mask_d = nc.dram_tensor("mask128", [128, 8 * 128], dt.float32, kind="ExternalInput").ap()
    tsq_d = nc.dram_tensor("tempsqrt", [128, B * HEADS], dt.float32, kind="ExternalInput").ap()
    out_d = nc.dram_tensor("out", [B, DIM, NV], dt.float32, kind="ExternalOutput").ap()

    NCHUNK = NV // 512       # 4 valid 512-chunks per batch
    PADW = HR * PITCH + 2    # 2342: padded row-pitch layout + 1 lead/trail col
    QCHUNK = [(i * 512, 512) for i in range(PADW // 512)] + \
             ([(PADW - PADW % 512, PADW % 512)] if PADW % 512 else [])
    QKTW = 2 * DIM + 32      # 800: transposed q|k tile width incl zero pad

    with tile.TileContext(nc) as tc:
        with tc.tile_pool(name="const", bufs=1) as const, \
             tc.tile_pool(name="work", bufs=2) as work, \
             tc.tile_pool(name="qkvp", bufs=2) as qkvp_pool, \
             tc.tile_pool(name="dwc", bufs=3) as dwc_pool, \
             tc.tile_pool(name="qkt", bufs=17) as qkt_pool, \
             tc.tile_pool(name="vres", bufs=1) as vres, \
             tc.tile_pool(name="small", bufs=3) as small, \
             tc.tile_pool(name="outp", bufs=1) as outp_pool, \
             tc.tile_pool(name="psum", bufs=2, space="PSUM") as psum, \
             tc.tile_pool(name="dram", bufs=1, space="DRAM") as dram:

            # ---- constants -------------------------------------------------
            wqkvT = [const.tile([128, 3 * DIM], dt.bfloat16, tag=f"wqkvT{k}", name=f"wqkvT{k}") for k in range(3)]
            for k in range(3):
                nc.sync.dma_start(wqkvT[k][:], wqkvT_d[k * 128:(k + 1) * 128, :])
            wdw = const.tile([128, 81], dt.float32, tag="wdw")
            nc.sync.dma_start(wdw[:], wdw_d[:])
            wprojT = [const.tile([128, DIM], dt.bfloat16, tag=f"wprojT{k}", name=f"wprojT{k}") for k in range(4)]
            for k in range(4):
                nc.sync.dma_start(wprojT[k][:], wprojT_d[k * 128:(k + 1) * 128, :])
            ident = const.tile([128, 128], dt.bfloat16, tag="ident")
            nc.sync.dma_start(ident[:], ident_d[:])
            id48 = const.tile([48, 48], dt.float32, tag="id48")
            nc.sync.dma_start(id48[:], id48_d[:])
            mask128 = const.tile([128, 8 * 128], dt.float32, tag="mask128")
            nc.sync.dma_start(mask128[:], mask_d[:])
            tsq = const.tile([128, B * HEADS], dt.float32, tag="tsq")
            nc.sync.dma_start(tsq[:], tsq_d[:])

            # diag(w_dw) stationary tiles: [ctile 0..9) x [tap 0..9)
            diags = []
            for ct in range(9):
                row = []
                for d in range(9):
                    dg = const.tile([128, 128], dt.bfloat16, tag=f"diag{ct}_{d}", name=f"diag{ct}_{d}")
                    nc.vector.tensor_scalar_mul(dg[:], ident[:], wdw[:, ct * 9 + d: ct * 9 + d + 1])
                    row.append(dg)
                diags.append(row)

            # persistent v (post-depthwise), bf16, head-pair layout
            v_sb = [[vres.tile([128, NV], dt.bfloat16, tag=f"v{b}_{hp}", name=f"v{b}_{hp}")
                     for hp in range(4)] for b in range(B)]

            # DRAM bounce buffers for the gram AllReduce
            gin = dram.tile([B, 128, 8 * 128], dt.float32, name="gin")
            gout = dram.tile([B, 128, 8 * 128], dt.float32, name="gout")

            # ---- phase A: per batch ---------------------------------------
            # x and qkv live in a padded spatial layout: position
            # 1 + r*130 + xp, with xp=0 / xp=129 zero pad columns, r = 0..17
            # slab rows incl halo.  Shifted depthwise reads are then plain
            # contiguous 1D slices (walrus: matmul rhs must be 1D-free).
            for b in range(B):
                x_sb = [work.tile([128, PADW], dt.bfloat16, tag=f"x{k}", name=f"x{k}")
                        for k in range(3)]
                for k in range(3):
                    xk3 = x_sb[k][:, 1:1 + HR * PITCH].rearrange(
                        "p (r w) -> p r w", w=PITCH)
                    nc.vector.memset(x_sb[k][:, 0:1], 0.0)
                    nc.vector.memset(xk3[:, :, 0:1], 0.0)
                    nc.vector.memset(xk3[:, :, PITCH - 1:PITCH], 0.0)
                    nc.sync.dma_start(
                        xk3[:, :, 1:1 + W],
                        x_d[b, k * 128:(k + 1) * 128, :].rearrange(
                            "p (r w) -> p r w", w=W))

                qkT = [qkt_pool.tile([128, QKTW], dt.bfloat16, tag="qkT", name=f"qkT{b}_{j}")
                       for j in range(ROWS)]
                for j in range(ROWS):
                    nc.vector.memset(qkT[j][:, 2 * DIM:], 0.0)

                for third in range(3):          # 0=q, 1=k, 2=v
                    for sub in range(3):        # channel tile within third
                        ot = third * 3 + sub
                        # -- 1x1 conv, computed directly in padded layout --
                        qkvp = qkvp_pool.tile([128, PADW], dt.bfloat16, tag="qkvp", name="qkvp")
                        for ci, (c0, cw) in enumerate(QCHUNK):
                            ps = psum.tile([128, 512], dt.float32, tag="mm512", bufs=3, name="ps")
                            for k in range(3):
                                nc.tensor.matmul(
                                    ps[:, :cw],
                                    lhsT=wqkvT[k][:, ot * 128:(ot + 1) * 128],
                                    rhs=x_sb[k][:, c0:c0 + cw],
                                    start=(k == 0), stop=(k == 2))
                            dst = qkvp[:, c0:c0 + cw]
                            if ci % 2 == 0:
                                nc.scalar.copy(dst, ps[:, :cw])
                            else:
                                nc.vector.tensor_copy(dst, ps[:, :cw])

                        # -- depthwise 3x3 via diagonal matmuls, 2 rows/chunk --
                        for ci in range(ROWS // 2):
                            # output rows r_in = 1+2ci, 2+2ci (input coords)
                            q0 = 1 + (1 + 2 * ci) * PITCH
                            ps = psum.tile([128, 2 * PITCH], dt.float32, tag="mm512", bufs=3, name="ps")
                            for d in range(9):
                                dy, dx = d // 3, d % 3
                                off = q0 + (dy - 1) * PITCH + (dx - 1)
                                nc.tensor.matmul(
                                    ps[:], lhsT=diags[ot][d][:],
                                    rhs=qkvp[:, off:off + 2 * PITCH],
                                    start=(d == 0), stop=(d == 8))
                            if third == 2:
                                # evict then DMA-repack (de-pad) into head-pair tiles
                                vstage = dwc_pool.tile([128, 2 * PITCH], dt.bfloat16, tag="dwc", name="vstage")
                                if ci % 2 == 0:
                                    nc.scalar.copy(vstage[:], ps[:])
                                else:
                                    nc.vector.tensor_copy(vstage[:], ps[:])
                                vs3 = vstage.rearrange("p (r w) -> p r w", w=PITCH)
                                for h in range(HEADS):
                                    lo = max(h * CH, sub * 128)
                                    hi = min(h * CH + CH, sub * 128 + 128)
                                    if lo >= hi:
                                        continue
                                    d0 = 64 * (h % 2) + (lo - h * CH)
                                    dst = v_sb[b][h // 2][
                                        d0:d0 + hi - lo,
                                        ci * 256:(ci + 1) * 256].rearrange(
                                            "p (r w) -> p r w", w=W)
                                    nc.sync.dma_start(
                                        dst,
                                        vs3[lo - sub * 128:hi - sub * 128, :, 1:1 + W])
                            else:
                                dwc = dwc_pool.tile([128, 2 * PITCH], dt.bfloat16, tag="dwc", name="dwc")
                                if ci % 2 == 0:
                                    nc.scalar.copy(dwc[:], ps[:])
                                else:
                                    nc.vector.tensor_copy(dwc[:], ps[:])
                                # -- transpose the 2 [128,128] image-row blocks --
                                pst = psum.tile([128, 256], dt.bfloat16, tag="tr", bufs=2, name="pst")
                                for j in range(2):
                                    nc.tensor.transpose(
                                        pst[:, j * 128:(j + 1) * 128],
                                        dwc[:, j * PITCH + 1:j * PITCH + 1 + W],
                                        ident[:])
                                col = third * DIM + sub * 128
                                for j in range(2):
                                    dst = qkT[ci * 2 + j][:, col:col + 128]
                                    src = pst[:, j * 128:(j + 1) * 128]
                                    if j % 2 == 0:
                                        nc.scalar.copy(dst, src)
                                    else:
                                        nc.vector.tensor_copy(dst, src)

                # -- per-head gram block [128, 128], built from 4 [64,64]
                # quadrant groups (matmul operands must be 1D-free 64-wide
                # windows of the transposed q|k tiles).  Block layout: rows /
                # cols [0:48) = q channels, [64:112) = k channels, i.e.
                # [Gqq | Gqk ; Gkq | Gkk] with 16-wide junk pads.
                for h in range(HEADS):
                    gstage = small.tile([128, 128], dt.float32, tag="gstage", name="gstage")
                    for qa in range(4):       # (lhs, rhs) in q/k x q/k
                        la, ra = qa // 2, qa % 2
                        pg = psum.tile([128, 64], dt.float32, tag="gram", bufs=2, name="pg")
                        for j in range(ROWS):
                            lhs = qkT[j][:, la * DIM + h * CH:la * DIM + h * CH + 64]
                            rhs = qkT[j][:, ra * DIM + h * CH:ra * DIM + h * CH + 64]
                            nc.tensor.matmul(pg[0:64, :], lhsT=lhs, rhs=rhs,
                                             start=(j == 0), stop=(j == ROWS - 1))
                        dst = gstage[la * 64:la * 64 + 64, ra * 64:ra * 64 + 64]
                        if qa % 2 == 0:
                            nc.scalar.copy(dst, pg[0:64, :])
                        else:
                            nc.vector.tensor_copy(dst, pg[0:64, :])
                    nc.sync.dma_start(gin[b][:, h * 128:(h + 1) * 128], gstage[:])

            # ---- AllReduce of grams ---------------------------------------
            nc.gpsimd.collective_compute(
                "AllReduce", mybir.AluOpType.add,
                replica_groups=[list(range(N_CORES))],
                ins=[gin.opt()], outs=[gout.opt()])

            # ---- norms: inv = sqrt(temp) / max(sqrt(diag(G)), eps) --------
            ssq = vres.tile([128, B * HEADS], dt.float32, tag="ssq", name="ssq")
            for b in range(B):
                gGn = work.tile([128, 8 * 128], dt.float32, tag="gG", name="gGn")
                nc.sync.dma_start(gGn[:], gout[b])
                tmp = work.tile([128, 8 * 128], dt.float32, tag="gtmp", name="gtmp", bufs=1)
                nc.vector.tensor_mul(tmp[:], gGn[:], mask128[:])
                t3 = tmp.rearrange("p (g c) -> p g c", c=128)
                nc.vector.tensor_reduce(
                    out=ssq[:, b * HEADS:(b + 1) * HEADS], in_=t3[:],
                    op=mybir.AluOpType.add, axis=mybir.AxisListType.X)
            inv = vres.tile([128, B * HEADS], dt.float32, tag="inv", name="inv")
            nc.scalar.activation(inv[:], ssq[:], mybir.ActivationFunctionType.Sqrt)
            nc.vector.tensor_scalar_max(inv[:], inv[:], EPS)
            nc.vector.reciprocal(inv[:], inv[:])
            nc.vector.tensor_mul(inv[:], inv[:], tsq[:])

            # ---- per (b, h): softmax pieces + attn@v; then projection -----
            for b in range(B):
                gGb = work.tile([128, 8 * 128], dt.float32, tag="gG", name="gGb")
                nc.sync.dma_start(gGb[:], gout[b])
                out_hp = [outp_pool.tile([128, NV], dt.bfloat16, tag=f"out{t}",
                                         name=f"out{t}", bufs=1) for t in range(4)]
                for t in range(4):
                    nc.vector.memset(out_hp[t][:], 0.0)
                for h in range(HEADS):
                    bh = b * HEADS + h
                    v0 = 64 * (h % 2)
                    g2 = gGb.rearrange("p (g c) -> p g c", c=128)
                    G_qk = g2[0:48, h, 64:112]     # [c, d]
                    G_kq = g2[64:112, h, 0:48]     # [d, c]
                    dq = small.tile([48, 48], dt.float32, tag="dq", name="dq")
                    nc.vector.tensor_scalar_mul(dq[:], id48[:], inv[0:48, bh:bh + 1])
                    dk_t = small.tile([128, 48], dt.float32, tag="dk", name="dk_t")
                    dk = dk_t[64:112, :]
                    nc.vector.tensor_scalar_mul(dk, id48[:], inv[64:112, bh:bh + 1])

                    # E_T[d,c] = exp(G_kq[d,c]*invq[c]*invk[d]), placed at the
                    # same base partition as the v slice (matmul base rule)
                    p1 = psum.tile([128, 128], dt.float32, tag="gram", bufs=2, name="p1")[0:48, 0:48]
                    nc.tensor.matmul(p1[:], lhsT=G_qk, rhs=dq[:], start=True, stop=True)
                    eT_t = small.tile([128, 48], dt.bfloat16, tag="eT", name="eT_t")
                    eT = eT_t[v0:v0 + CH, :]
                    nc.scalar.activation(eT, p1[:], mybir.ActivationFunctionType.Exp,
                                         scale=inv[64:112, bh:bh + 1])
                    # E[c,d] (for Z only) with fused row-sum
                    p2 = psum.tile([128, 128], dt.float32, tag="gram", bufs=2, name="p2")[0:48, 0:48]
                    nc.tensor.matmul(p2[:], lhsT=G_kq, rhs=dk, start=True, stop=True)
                    escr = small.tile([48, 48], dt.float32, tag="escr", name="escr")
                    zt = small.tile([48, 1], dt.float32, tag="zt", name="zt")
                    nc.scalar.activation(escr[:], p2[:], mybir.ActivationFunctionType.Exp,
                                         scale=inv[0:48, bh:bh + 1], accum_out=zt[:])
                    invz = small.tile([48, 1], dt.float32, tag="invz", name="invz")
                    nc.vector.reciprocal(invz[:], zt[:])

                    for ci in range(NCHUNK):
                        po = psum.tile([128, 512], dt.float32, tag="mm512", bufs=3, name="po")[0:48, :]
                        nc.tensor.matmul(
                            po[:], lhsT=eT,
                            rhs=v_sb[b][h // 2][v0:v0 + CH, ci * 512:(ci + 1) * 512],
                            start=True, stop=True)
                        nc.scalar.activation(
                            out_hp[h // 2][v0:v0 + CH, ci * 512:(ci + 1) * 512],
                            po[:], mybir.ActivationFunctionType.Copy,
                            scale=invz[:])

                # -- projection (K-tiles are the 4 head-pair tiles) --
                for ot in range(3):
                    for ci in range(NCHUNK):
                        pp = psum.tile([128, 512], dt.float32, tag="mm512", bufs=3, name="pp")
                        for k in range(4):
                            nc.tensor.matmul(
                                pp[:], lhsT=wprojT[k][:, ot * 128:(ot + 1) * 128],
                                rhs=out_hp[k][:, ci * 512:(ci + 1) * 512],
                                start=(k == 0), stop=(k == 3))
                        res = work.tile([128, 512], dt.float32, tag="res", name="res")
                        if ci % 2 == 0:
                            nc.scalar.copy(res[:], pp[:])
                        else:
                            nc.vector.tensor_copy(res[:], pp[:])
                        nc.sync.dma_start(
                            out_d[b, ot * 128:(ot + 1) * 128, ci * 512:(ci + 1) * 512],
                            res[:])

    nc.compile()
    return nc


def _get_nc():
    if "nc" not in _CACHE:
        _CACHE["nc"] = _build()
    return _CACHE["nc"]


def _prep_inputs(x, w_qkv, w_dw, w_proj, temperature):
    x = np.asarray(x, np.float32)
    w_qkv = np.asarray(w_qkv, np.float32)
    w_dw = np.asarray(w_dw, np.float32).reshape(3 * DIM, 9)
    w_proj = np.asarray(w_proj, np.float32)
    temperature = np.asarray(temperature, np.float32).reshape(HEADS)

    # halo-padded x slabs, bf16: [core][B, DIM, HR*W]
    xp = np.zeros((B, DIM, H + 2, W), np.float32)
    xp[:, :, 1:H + 1, :] = x
    xs = []
    for i in range(N_CORES):
        sl = xp[:, :, i * ROWS:i * ROWS + HR, :].reshape(B, DIM, NH)
        xs.append(sl.astype(BF16))

    wqkvT = np.ascontiguousarray(w_qkv.T).astype(BF16)          # [384, 1152]
    wdw = np.empty((128, 81), np.float32)
    for ct in range(9):
        for d in range(9):
            wdw[:, ct * 9 + d] = w_dw[ct * 128:(ct + 1) * 128, d]
    # projection weights in head-pair row layout: tile hp row p ->
    # attention-output channel 96*hp + p (p<48) / 96*hp + 48 + (p-64)
    wprojT_hp = np.zeros((4 * 128, DIM), np.float32)
    for hp in range(4):
        wprojT_hp[hp * 128 + 0:hp * 128 + 48, :] = w_proj.T[96 * hp: 96 * hp + 48, :]
        wprojT_hp[hp * 128 + 64:hp * 128 + 112, :] = w_proj.T[96 * hp + 48: 96 * hp + 96, :]
    wprojT_hp = wprojT_hp.astype(BF16)
    ident = np.eye(128, dtype=np.float32).astype(BF16)
    id48 = np.eye(48, dtype=np.float32)
    mask128 = np.tile(np.eye(128, dtype=np.float32), (1, 8))    # [128, 1024]
    tsq = np.empty((128, B * HEADS), np.float32)
    for b in range(B):
        for h in range(HEADS):
            tsq[:, b * HEADS + h] = np.sqrt(max(temperature[h], 0.0))

    common = dict(wqkvT=wqkvT, wdw=wdw, wprojT=wprojT_hp, ident=ident,
                  id48=id48, mask128=mask128, tempsqrt=tsq)
    return [dict(common, x=xs[i]) for i in range(N_CORES)]


def run_device(in_maps, **kw):
    from concourse.bass_utils import run_bass_kernel_spmd
    nc = _get_nc()
    return run_bass_kernel_spmd(nc, in_maps, list(range(N_CORES)), **kw)


def kernel(x, w_qkv, w_dw, w_proj, temperature):
    in_maps = _prep_inputs(x, w_qkv, w_dw, w_proj, temperature)
    res = run_device(in_maps)
    full = np.empty((B, DIM, H, W), np.float32)
    for i in range(N_CORES):
        full[:, :, i * ROWS:(i + 1) * ROWS, :] = \
            res.results[i]["out"].reshape(B, DIM, ROWS, W)
    return full


# revision 37
# speedup vs baseline: 11522.8505x; 11522.8505x over previous
"""nn_Attention_50749333569807 — 8-core Trainium2 Bass kernel.

Channel attention (XCA-style) over x[4, 384, 128, 128]:
  qkv = 1x1 conv -> depthwise 3x3 -> per-head (8) channel attention with
  L2-normalized q,k over the spatial axis -> 1x1 projection.

Sharding: H (128 rows) split into 8 slabs of 16 rows, one per NeuronCore.
All 4 batches + all channels live on every core.  The depthwise conv uses a
1-row halo (host-supplied, zero padded at image edges).  The only cross-core
coupling is the spatial contraction inside the q/k grams and norms; that is
one AllReduce of the 32 per-(batch,head) gram blocks (2 MB).

Layout notes (HW: engine APs need partition base % 32 == 0; matmul operands
need equal bases in {0,32,64}):
  * gram blocks are [128,128]: q channels at rows/cols [0,48), k at [64,112),
    by reading 64-wide per-head windows of the transposed q|k tiles.
  * v and the attention output use a "head pair" layout: tile hp holds head
    2hp at partitions [0,48) and head 2hp+1 at [64,112).  v is repacked from
    the natural eviction layout via partition-offset DMAs (DMA has no
    alignment restriction); the projection weights are host-permuted to
    match, with zero rows at the padding.

Per-core pipeline (per batch):
  1. qkv GEMM  [1152,384]@[384,2304]  (bf16, f32 psum), evicted into a
     width-130 padded layout (explicit zero columns for x-edge padding).
  2. depthwise 3x3 = 9 accumulating matmuls with diagonal-stationary
     weights against shifted slices of the padded qkv.
  3. q,k tiles PE-transposed to [n, c] layout (bf16).
  4. gram[h] accumulated over 16 K-tiles of 128 pixels -> staged to DRAM.
  -- AllReduce over 8 cores --
  5. norms from the gram diagonal; softmax scaling folded as
     attn = softmax(G_qk * inv_q[c] * inv_k[d] * temp); 1/Z folded into
     the attn@v eviction.
  6. out = attn @ v per head; final = w_proj @ out; DMA to output.
"""

import sys

import numpy as np

if "/opt/trn_rl_repo" not in sys.path:
    sys.path.insert(0, "/opt/trn_rl_repo")

import ml_dtypes

BF16 = ml_dtypes.bfloat16

B, DIM, H, W = 4, 384, 128, 128
HEADS = 8
CH = DIM // HEADS            # 48
N_CORES = 8
ROWS = H // N_CORES          # 16 valid rows per core
HR = ROWS + 2                # 18 rows incl halo
PITCH = W + 2                # 130 padded row pitch
NV = ROWS * W                # 2048 valid pixels / core / batch
NH = HR * W                  # 2304 pixels incl halo
EPS = 1e-12

_CACHE = {}


def _build(with_collective=True, iters=1):
    import concourse.tile as tile
    from concourse import bacc, mybir

    dt = mybir.dt
    nc = bacc.Bacc("TRN2", target_bir_lowering=False, debug=False,
                   num_devices=N_CORES)

    x_d = nc.dram_tensor("x", [B, DIM, NH], dt.bfloat16, kind="ExternalInput").ap()
    wqkvT_d = nc.dram_tensor("wqkvT", [DIM, 3 * DIM], dt.bfloat16, kind="ExternalInput").ap()
    diags_d = nc.dram_tensor("diags", [128, 81 * 128], dt.bfloat16, kind="ExternalInput").ap()
    wprojT_d = nc.dram_tensor("wprojT", [4 * 128, DIM], dt.bfloat16, kind="ExternalInput").ap()
    ident_d = nc.dram_tensor("ident", [128, 128], dt.bfloat16, kind="ExternalInput").ap()
    id48_d = nc.dram_tensor("id48", [48, 48], dt.float32, kind="ExternalInput").ap()
    mask_d = nc.dram_tensor("mask128", [128, 8 * 128], dt.bfloat16, kind="ExternalInput").ap()
    tsq_d = nc.dram_tensor("tempsqrt", [128, B * HEADS], dt.float32, kind="ExternalInput").ap()
    out_d = nc.dram_tensor("out", [B, DIM, NV], dt.float32, kind="ExternalOutput").ap()

    NCHUNK = NV // 512       # 4 valid 512-chunks per batch
    PADW = HR * PITCH + 2    # 2342: padded row-pitch layout + 1 lead/trail col
    QCHUNK = [(i * 512, 512) for i in range(PADW // 512)] + \
             ([(PADW - PADW % 512, PADW % 512)] if PADW % 512 else [])
    QKTW = 2 * DIM + 32      # 800: transposed q|k tile width incl zero pad

    with tile.TileContext(nc) as tc:
        with tc.tile_pool(name="const", bufs=1) as const, \
             tc.tile_pool(name="work", bufs=2) as work, \
             tc.tile_pool(name="qkvp", bufs=2) as qkvp_pool, \
             tc.tile_pool(name="dwc", bufs=3) as dwc_pool, \
             tc.tile_pool(name="qkt", bufs=16) as qkt_pool, \
             tc.tile_pool(name="vres", bufs=1) as vres, \
             tc.tile_pool(name="small", bufs=4) as small, \
             tc.tile_pool(name="outp", bufs=1) as outp_pool, \
             tc.tile_pool(name="psum", bufs=2, space="PSUM") as psum, \
             tc.tile_pool(name="dram", bufs=1, space="DRAM") as dram:

            # ---- constants -------------------------------------------------
            wqkvT = [const.tile([128, 3 * DIM], dt.bfloat16, tag=f"wqkvT{k}", name=f"wqkvT{k}") for k in range(3)]
            for k in range(3):
                nc.sync.dma_start(wqkvT[k][:], wqkvT_d[k * 128:(k + 1) * 128, :])
            wprojT = [const.tile([128, DIM], dt.bfloat16, tag=f"wprojT{k}", name=f"wprojT{k}") for k in range(4)]
            for k in range(4):
                nc.sync.dma_start(wprojT[k][:], wprojT_d[k * 128:(k + 1) * 128, :])
            ident = const.tile([128, 128], dt.bfloat16, tag="ident")
            nc.sync.dma_start(ident[:], ident_d[:])
            id48 = const.tile([48, 48], dt.float32, tag="id48")
            nc.sync.dma_start(id48[:], id48_d[:])
            mask128 = const.tile([128, 8 * 128], dt.bfloat16, tag="mask128")
            nc.sync.dma_start(mask128[:], mask_d[:])
            tsq = const.tile([128, B * HEADS], dt.float32, tag="tsq")
            nc.sync.dma_start(tsq[:], tsq_d[:])

            # diag(w_dw) stationary tiles, host-prebuilt: one DMA, sliced
            # as [ctile 0..9) x [tap 0..9)
            diag_big = const.tile([128, 81 * 128], dt.bfloat16, tag="diag_big")
            nc.sync.dma_start(diag_big[:], diags_d[:])
            diags = [[diag_big[:, (ct * 9 + d) * 128:(ct * 9 + d + 1) * 128]
                      for d in range(9)] for ct in range(9)]

            # persistent v (post-depthwise), bf16, head-pair layout
            v_sb = [[vres.tile([128, NV], dt.bfloat16, tag=f"v{b}_{hp}", name=f"v{b}_{hp}")
                     for hp in range(4)] for b in range(B)]

            # DRAM bounce buffers for the gram AllReduce
            gin = dram.tile([B, 128, 8 * 128], dt.float32, name="gin")
            gout = dram.tile([B, 128, 8 * 128], dt.float32, name="gout")

            # ---- phase A: per batch ---------------------------------------
            # x and qkv live in a padded spatial layout: position
            # 1 + r*130 + xp, with xp=0 / xp=129 zero pad columns, r = 0..17
            # slab rows incl halo.  Shifted depthwise reads are then plain
            # contiguous 1D slices (walrus: matmul rhs must be 1D-free).
            def phase_a(b):
                # zero the junk partition bands of the head-pair v tiles so a
                # K=112 window in the final GEMM sees 0 * 0 there
                for hp in range(4):
                    nc.vector.memset(v_sb[b][hp][32:64, :], 0.0)
                    nc.vector.memset(v_sb[b][hp][96:128, :], 0.0)
                x_sb = [work.tile([128, PADW], dt.bfloat16, tag=f"x{k}",
                                  name=f"x{k}", bufs=2) for k in range(3)]
                xk3s = []
                for k in range(3):
                    xk3 = x_sb[k][:, 1:1 + HR * PITCH].rearrange(
                        "p (r w) -> p r w", w=PITCH)
                    nc.vector.memset(x_sb[k][:, 0:1], 0.0)
                    nc.vector.memset(xk3[:, :, 0:1], 0.0)
                    nc.vector.memset(xk3[:, :, PITCH - 1:PITCH], 0.0)
                    xk3s.append(xk3)
                # row-group-major DMA order: the first qkv chunks need the
                # leading rows of ALL three channel tiles
                for rr in range(0, HR, 6):
                    nr = min(6, HR - rr)
                    for k in range(3):
                        nc.sync.dma_start(
                            xk3s[k][:, rr:rr + nr, 1:1 + W],
                            x_d[b, k * 128:(k + 1) * 128,
                                rr * W:(rr + nr) * W].rearrange(
                                "p (r w) -> p r w", w=W))

                qkT = [qkt_pool.tile([128, QKTW], dt.bfloat16, tag="qkT", name=f"qkT{b}_{j}")
                       for j in range(ROWS)]
                for j in range(ROWS):
                    nc.vector.memset(qkT[j][:, 2 * DIM:], 0.0)

                for third in (0, 1, 2):         # q, k, v
                    for sub in range(3):        # channel tile within third
                        ot = third * 3 + sub
                        # -- 1x1 conv, computed directly in padded layout --
                        qkvp = qkvp_pool.tile([128, PADW], dt.bfloat16, tag="qkvp", name="qkvp", bufs=3)
                        for ci, (c0, cw) in enumerate(QCHUNK):
                            ps = psum.tile([128, 512], dt.float32, tag="mm512", bufs=4, name="ps")
                            for k in range(3):
                                nc.tensor.matmul(
                                    ps[:, :cw],
                                    lhsT=wqkvT[k][:, ot * 128:(ot + 1) * 128],
                                    rhs=x_sb[k][:, c0:c0 + cw],
                                    start=(k == 0), stop=(k == 2))
                            dst = qkvp[:, c0:c0 + cw]
                            if ci % 2 == 0:
                                nc.scalar.copy(dst, ps[:, :cw])
                            else:
                                nc.vector.tensor_copy(dst, ps[:, :cw])

                        # -- depthwise 3x3 via diagonal matmuls --
                        # output positions live in the same padded layout;
                        # the valid window is rows 1..17 = [PITCH+1, PITCH+1
                        # + DWN).  512-col chunks stream long matmuls; evicts
                        # land in a full padded staging tile.
                        DWN = ROWS * PITCH  # 2080 padded output positions
                        q0 = PITCH + 1
                        dwf = dwc_pool.tile([128, DWN], dt.bfloat16, tag="dwc", name="dwf", bufs=2)
                        for ci, (c0, cw) in enumerate(
                                [(i * 512, min(512, DWN - i * 512))
                                 for i in range((DWN + 511) // 512)]):
                            ps = psum.tile([128, 512], dt.float32, tag="mm512", bufs=4, name="ps")
                            for d in range(9):
                                dy, dx = d // 3, d % 3
                                off = q0 + c0 + (dy - 1) * PITCH + (dx - 1)
                                nc.tensor.matmul(
                                    ps[:, :cw], lhsT=diags[ot][d][:],
                                    rhs=qkvp[:, off:off + cw],
                                    start=(d == 0), stop=(d == 8))
                            if ci % 2 == 0:
                                nc.scalar.copy(dwf[:, c0:c0 + cw], ps[:, :cw])
                            else:
                                nc.vector.tensor_copy(dwf[:, c0:c0 + cw], ps[:, :cw])
                        # dwf position p maps to padded coords (row 1+p//130,
                        # xp p%130): image row j at [j*PITCH+1, +W)
                        if third == 2:
                            # DMA-repack (de-pad) into head-pair v tiles
                            for h in range(HEADS):
                                lo = max(h * CH, sub * 128)
                                hi = min(h * CH + CH, sub * 128 + 128)
                                if lo >= hi:
                                    continue
                                d0 = 64 * (h % 2) + (lo - h * CH)
                                dst = v_sb[b][h // 2][d0:d0 + hi - lo, :].rearrange(
                                    "p (r w) -> p r w", w=W)
                                src = dwf[lo - sub * 128:hi - sub * 128, :].rearrange(
                                    "p (r w) -> p r w", w=PITCH)[:, :, 1:1 + W]
                                nc.sync.dma_start(dst, src)
                        else:
                            # -- transpose 16 [128,128] image-row blocks --
                            col = third * DIM + sub * 128
                            for jj in range(ROWS // 4):
                                pst = psum.tile([128, 512], dt.bfloat16, tag="tr", bufs=2, name="pst")
                                for j4 in range(4):
                                    j = jj * 4 + j4
                                    nc.tensor.transpose(
                                        pst[:, j4 * 128:(j4 + 1) * 128],
                                        dwf[:, j * PITCH + 1:j * PITCH + 1 + W],
                                        ident[:])
                                for j4 in range(4):
                                    j = jj * 4 + j4
                                    dst = qkT[j][:, col:col + 128]
                                    src = pst[:, j4 * 128:(j4 + 1) * 128]
                                    if j4 % 2 == 0:
                                        nc.scalar.copy(dst, src)
                                    else:
                                        nc.vector.tensor_copy(dst, src)

                # -- per-head gram block [128, 128], built from 4 [64,64]
                # quadrant groups (matmul operands must be 1D-free 64-wide
                # windows of the transposed q|k tiles).  Block layout: rows /
                # cols [0:48) = q channels, [64:112) = k channels, i.e.
                # [Gqq | Gqk ; Gkq | Gkk] with 16-wide junk pads.
                for h in range(HEADS):
                    gstage = small.tile([128, 128], dt.float32, tag="gstage", name="gstage")
                    nc.vector.memset(gstage[:], 0.0)
                    for la, ra in [(0, 0), (1, 0), (1, 1)]:   # qq, kq, kk
                        pg = psum.tile([128, 64], dt.float32, tag="gram", bufs=2, name="pg")
                        for j in range(ROWS):
                            lhs = qkT[j][:, la * DIM + h * CH:la * DIM + h * CH + CH]
                            rhs = qkT[j][:, ra * DIM + h * CH:ra * DIM + h * CH + CH]
                            nc.tensor.matmul(pg[0:CH, :CH], lhsT=lhs, rhs=rhs,
                                             start=(j == 0), stop=(j == ROWS - 1))
                        dst = gstage[la * 64:la * 64 + CH, ra * 64:ra * 64 + CH]
                        if (la + ra) % 2 == 0:
                            nc.scalar.copy(dst, pg[0:CH, :CH])
                        else:
                            nc.vector.tensor_copy(dst, pg[0:CH, :CH])
                    nc.sync.dma_start(gin[b][:, h * 128:(h + 1) * 128], gstage[:])

                # per-batch AllReduce so post-AR work for early batches
                # overlaps phase A of later ones
                if with_collective:
                    nc.gpsimd.collective_compute(
                        "AllReduce", mybir.AluOpType.add,
                        replica_groups=[list(range(N_CORES))],
                        ins=[gin[b]], outs=[gout[b]])
                else:
                    nc.sync.dma_start(gout[b], gin[b])

            # ---- per (b, h): norms, softmax pieces, attn@v, projection ----
            def post_ar(b):
                gGb = work.tile([128, 8 * 128], dt.float32, tag="gG", name="gGb")
                nc.sync.dma_start(gGb[:], gout[b])
                # norms: inv = sqrt(temp) / max(sqrt(diag(G)), eps)
                tmp = work.tile([128, 8 * 128], dt.float32, tag="gtmp", name="gtmp", bufs=1)
                nc.vector.tensor_mul(tmp[:], gGb[:], mask128[:])
                t3 = tmp.rearrange("p (g c) -> p g c", c=128)
                inv = small.tile([128, HEADS], dt.float32, tag="inv", name="inv", bufs=2)
                nc.vector.tensor_reduce(
                    out=inv[:], in_=t3[:],
                    op=mybir.AluOpType.add, axis=mybir.AxisListType.X)
                # rsqrt via bit-trick seed + 2 Newton steps, all on DVE --
                # avoids an ACT Sqrt whose table set would evict Exp's
                # (one table reload costs ~2.7us of ScalarE time per batch).
                nw1 = small.tile([128, HEADS], dt.float32, tag="nw1", name="nw1", bufs=2)
                nw2 = small.tile([128, HEADS], dt.float32, tag="nw2", name="nw2", bufs=2)
                nc.vector.tensor_scalar(
                    out=nw2.bitcast(dt.int32), in0=inv.bitcast(dt.int32),
                    scalar1=1, scalar2=None, op0=mybir.AluOpType.arith_shift_right)
                nc.vector.tensor_copy(nw1[:], nw2.bitcast(dt.int32))  # int -> f32
                nc.vector.tensor_scalar(
                    out=nw1[:], in0=nw1[:], scalar1=-1.0, scalar2=float(0x5F3759DF),
                    op0=mybir.AluOpType.mult, op1=mybir.AluOpType.add)
                nc.vector.tensor_copy(nw2.bitcast(dt.int32), nw1[:])  # f32 -> int
                for _ in range(2):
                    nc.vector.tensor_mul(nw1[:], inv[:], nw2[:])      # x*y
                    nc.vector.tensor_mul(nw1[:], nw1[:], nw2[:])      # x*y^2
                    nc.vector.tensor_scalar(
                        out=nw1[:], in0=nw1[:], scalar1=-0.5, scalar2=1.5,
                        op0=mybir.AluOpType.mult, op1=mybir.AluOpType.add)
                    nc.vector.tensor_mul(nw2[:], nw2[:], nw1[:])      # y *= h
                nc.vector.tensor_scalar_min(nw2[:], nw2[:], 1.0 / EPS)
                nc.vector.tensor_mul(inv[:], nw2[:], tsq[:, b * HEADS:(b + 1) * HEADS])
                # merged attention+projection: per head build
                # M_h^T[d, o] = (E diag(1/Z))^T @ w_projT rows of head h,
                # then final[o, n] = sum_h M_h^T.T @ v_h in one fused GEMM.
                Mt = [outp_pool.tile([128, DIM], dt.bfloat16, tag=f"Mt{t}",
                                     name=f"Mt{t}", bufs=2) for t in range(4)]
                for t in range(4):
                    nc.vector.memset(Mt[t][32:64, :], 0.0)
                    nc.vector.memset(Mt[t][96:128, :], 0.0)
                for h in range(HEADS):
                    v0 = 64 * (h % 2)
                    g2 = gGb.rearrange("p (g c) -> p g c", c=128)
                    G_kq = g2[64:112, h, 0:48]     # [d, c]
                    dk_t = small.tile([128, 48], dt.float32, tag="dk", name="dk_t")
                    dk = dk_t[64:112, :]
                    nc.vector.tensor_scalar_mul(dk, id48[:], inv[64:112, h:h + 1])

                    # E[c,d] = exp(G_qk*invq*invk) with fused row-sum Z_c
                    p2 = psum.tile([128, 128], dt.float32, tag="gram", bufs=2, name="p2")[0:48, 0:48]
                    nc.tensor.matmul(p2[:], lhsT=G_kq, rhs=dk, start=True, stop=True)
                    escr = small.tile([48, 48], dt.float32, tag="escr", name="escr")
                    zt = small.tile([48, 1], dt.float32, tag="zt", name="zt")
                    nc.scalar.activation(escr[:], p2[:], mybir.ActivationFunctionType.Exp,
                                         scale=inv[0:48, h:h + 1], accum_out=zt[:])
                    invz = small.tile([48, 1], dt.float32, tag="invz", name="invz")
                    nc.vector.reciprocal(invz[:], zt[:])
                    # attn rows scaled by 1/Z, placed at the v slice's base
                    Es_t = small.tile([128, 48], dt.bfloat16, tag="eT", name="Es_t")
                    Es = Es_t[v0:v0 + CH, :]
                    nc.vector.tensor_scalar_mul(Es, escr[:], invz[:])

                    pm = psum.tile([128, DIM], dt.float32, tag="tr", bufs=2, name="pm")[0:48, :]
                    nc.tensor.matmul(pm[:], lhsT=Es,
                                     rhs=wprojT[h // 2][v0:v0 + CH, :],
                                     start=True, stop=True)
                    dst = Mt[h // 2][v0:v0 + CH, :]
                    if h % 2 == 0:
                        nc.scalar.copy(dst, pm[:])
                    else:
                        nc.vector.tensor_copy(dst, pm[:])

                # -- fused final GEMM: one K=112 window per head-pair tile
                # (zeroed junk bands make the extra rows no-ops) --
                for ot in range(3):
                    for ci in range(NCHUNK):
                        pp = psum.tile([128, 512], dt.float32, tag="mm512", bufs=4, name="pp")
                        for hp in range(4):
                            nc.tensor.matmul(
                                pp[:],
                                lhsT=Mt[hp][0:112, ot * 128:(ot + 1) * 128],
                                rhs=v_sb[b][hp][0:112, ci * 512:(ci + 1) * 512],
                                start=(hp == 0), stop=(hp == 3))
                        res = work.tile([128, 512], dt.float32, tag="res", name="res")
                        if ci % 2 == 0:
                            nc.scalar.copy(res[:], pp[:])
                        else:
                            nc.vector.tensor_copy(res[:], pp[:])
                        nc.sync.dma_start(
                            out_d[b, ot * 128:(ot + 1) * 128, ci * 512:(ci + 1) * 512],
                            res[:])

            # ---- emission schedule ----
            # iters > 1 replays the whole computation (used by test.py to
            # measure pure on-device time as a wall-clock delta)
            for _ in range(iters):
                for b in range(B):
                    phase_a(b)
                for b in range(B):
                    post_ar(b)

    nc.compile()
    return nc


def _get_nc():
    if "nc" not in _CACHE:
        _CACHE["nc"] = _build()
    return _CACHE["nc"]


def timeline_report(save_trace=None):
    """Cost-model (TimelineSim) duration + per-engine busy summary for a
    single-core, collective-free variant of the kernel."""
    from concourse.timeline_sim import TimelineSim
    nc = _build(with_collective=False)
    tl = TimelineSim(nc, trace=save_trace is not None)
    tl.simulate()
    total = tl.time
    if save_trace:
        tl.perfetto.save(save_trace)
    return total


def _prep_inputs(x, w_qkv, w_dw, w_proj, temperature):
    x = np.asarray(x, np.float32)
    w_qkv = np.asarray(w_qkv, np.float32)
    w_dw = np.asarray(w_dw, np.float32).reshape(3 * DIM, 9)
    w_proj = np.asarray(w_proj, np.float32)
    temperature = np.asarray(temperature, np.float32).reshape(HEADS)

    # halo-padded x slabs, bf16: [core][B, DIM, HR*W]
    xp = np.zeros((B, DIM, H + 2, W), np.float32)
    xp[:, :, 1:H + 1, :] = x
    xs = []
    for i in range(N_CORES):
        sl = xp[:, :, i * ROWS:i * ROWS + HR, :].reshape(B, DIM, NH)
        xs.append(sl.astype(BF16))

    wqkvT = np.ascontiguousarray(w_qkv.T).astype(BF16)          # [384, 1152]
    diags = np.zeros((128, 81 * 128), np.float32)
    ii = np.arange(128)
    for ct in range(9):
        for d in range(9):
            diags[ii, (ct * 9 + d) * 128 + ii] = w_dw[ct * 128:(ct + 1) * 128, d]
    diags = diags.astype(BF16)
    # projection weights in head-pair row layout: tile hp row p ->
    # attention-output channel 96*hp + p (p<48) / 96*hp + 48 + (p-64)
    wprojT_hp = np.zeros((4 * 128, DIM), np.float32)
    for hp in range(4):
        wprojT_hp[hp * 128 + 0:hp * 128 + 48, :] = w_proj.T[96 * hp: 96 * hp + 48, :]
        wprojT_hp[hp * 128 + 64:hp * 128 + 112, :] = w_proj.T[96 * hp + 48: 96 * hp + 96, :]
    wprojT_hp = wprojT_hp.astype(BF16)
    ident = np.eye(128, dtype=np.float32).astype(BF16)
    id48 = np.eye(48, dtype=np.float32)
    mask128 = np.tile(np.eye(128, dtype=np.float32), (1, 8)).astype(BF16)
    tsq = np.empty((128, B * HEADS), np.float32)
    for b in range(B):
        for h in range(HEADS):
            tsq[:, b * HEADS + h] = np.sqrt(max(temperature[h], 0.0))

    common = dict(wqkvT=wqkvT, diags=diags, wprojT=wprojT_hp, ident=ident,
                  id48=id48, mask128=mask128, tempsqrt=tsq)
    return [dict(common, x=xs[i]) for i in range(N_CORES)]


def run_device(in_maps, **kw):
    from concourse.bass_utils import run_bass_kernel_spmd
    nc = _get_nc()
    return run_bass_kernel_spmd(nc, in_maps, list(range(N_CORES)), **kw)


def kernel(x, w_qkv, w_dw, w_proj, temperature):
    in_maps = _prep_inputs(x, w_qkv, w_dw, w_proj, temperature)
    res = run_device(in_maps)
    full = np.empty((B, DIM, H, W), np.float32)
    for i in range(N_CORES):
        full[:, :, i * ROWS:(i + 1) * ROWS, :] = \
            res.results[i]["out"].reshape(B, DIM, ROWS, W)
    return full
